# revision 2
# baseline (speedup 1.0000x reference)
"""Trainium2 Bass kernel for a 2-layer GRU encoder (nn_Encoder_28028956574172).

Gate-major redesign (v2): weights are the matmul STATIONARY operand and the
transposed hidden state [h, batch] is the 64-wide MOVING operand, so gates
land directly in [gate, batch] layout. This
  - eliminates all per-step PE transposes (state is produced in-layout),
  - runs every matmul in bf16 (1 cycle/row at any moving size),
  - puts activations/elementwise on full 128-partition tiles (halves ACT/DVE
    element counts vs batch-major),
  - shortens the per-step cross-engine dependency cycle.

Per-step structure (super-step t = L1 GRU step t + L2 GRU step t-1):
  PSUM banks (one per region, double-buffered across step parity):
    rz1 [128,256]: r1 chunks (cols 0:128), z1 chunks (128:256)
    n1  [128,256]: i_n1 (0:128), h_n1 (128:256)
    rz2, n2: same for layer 2
  ACT: r = sigmoid(rz r-half); w = sigmoid(-(z-half)) = 1-z (scale=-1);
       n = tanh(v)
  v = i_n + r*h_n is formed by a PE identity-matmul that ACCUMULATES
  u = r*h_n (DVE product) into the i_n PSUM region (saves a DVE hop).
  Update uses h' = w*n + (h - w*h): q=w*h and p=h-q run OFF the critical
  chain (only m=w*n and h'=m+p are on it).

Biases: L1 ride a ones-row appended to the input (row 80); b_hh1n and all
L2 biases are injected by tiny K<=4 "pattern" matmuls into PSUM.
"""

import numpy as np
from ml_dtypes import bfloat16

import concourse.bacc as bacc
import concourse.bass as bass
import concourse.mybir as mybir
import concourse.tile as tile
from concourse import bass_utils

F32 = mybir.dt.float32
BF16 = mybir.dt.bfloat16

B, S, DIN, DC, H, REP = 512, 1024, 64, 16, 256, 128
NCORES = 8
BL = B // NCORES          # 64 batch per core
DXA = DIN + DC + 1        # 81: input+cond+ones row
CHUNK = 128               # timesteps per input DMA chunk
NCHUNKS = S // CHUNK      # 8
NSUPER = S + 1


def build_program(n_super=NSUPER, dumps=False):
    nc = bacc.Bacc(
        "TRN2",
        target_bir_lowering=False,
        debug=False,
        enable_asserts=False,
        num_devices=NCORES,
    )

    # ---- DRAM I/O (bf16 unless noted) ----
    xt_d = nc.dram_tensor("xt", [DXA, S, BL], BF16, kind="ExternalInput")
    wgi1_d = nc.dram_tensor("wgi1", [DXA, 768], BF16, kind="ExternalInput")
    wgh1rz_d = nc.dram_tensor("wgh1rz", [2, 128, 512], BF16, kind="ExternalInput")
    wgh1n_d = nc.dram_tensor("wgh1n", [2, 128, 256], BF16, kind="ExternalInput")
    bhh1n_d = nc.dram_tensor("bhh1n", [2, 128], BF16, kind="ExternalInput")
    wgi2rz_d = nc.dram_tensor("wgi2rz", [2, 128, 512], BF16, kind="ExternalInput")
    wgi2n_d = nc.dram_tensor("wgi2n", [2, 128, 256], BF16, kind="ExternalInput")
    wgh2rz_d = nc.dram_tensor("wgh2rz", [2, 128, 512], BF16, kind="ExternalInput")
    wgh2n_d = nc.dram_tensor("wgh2n", [2, 128, 256], BF16, kind="ExternalInput")
    brz2_d = nc.dram_tensor("brz2", [4, 128], BF16, kind="ExternalInput")
    bin2_d = nc.dram_tensor("bin2", [2, 128], BF16, kind="ExternalInput")
    bhn2_d = nc.dram_tensor("bhn2", [2, 128], BF16, kind="ExternalInput")
    wlin_d = nc.dram_tensor("wlin", [2, 128, REP], BF16, kind="ExternalInput")
    blin_d = nc.dram_tensor("blin", [1, REP], BF16, kind="ExternalInput")
    ident_d = nc.dram_tensor("ident", [128, 128], BF16, kind="ExternalInput")
    pat2_d = nc.dram_tensor("pat2", [2, 128], BF16, kind="ExternalInput")
    pat4_d = nc.dram_tensor("pat4", [4, 256], BF16, kind="ExternalInput")
    ones_d = nc.dram_tensor("ones", [1, BL], BF16, kind="ExternalInput")
    out_d = nc.dram_tensor("outT", [REP, BL], F32, kind="ExternalOutput")

    AF = mybir.ActivationFunctionType

    with tile.TileContext(nc) as tc:
        with (
            tc.tile_pool(name="wpool", bufs=1) as wp,
            tc.tile_pool(name="xpool", bufs=3) as xp,
            tc.tile_pool(name="state", bufs=2) as sp,
            tc.tile_pool(name="work", bufs=2) as wk,
            tc.tile_pool(name="psum", bufs=2, space=bass.MemorySpace.PSUM) as gp,
        ):
            # ---- resident weights ----
            def wtile(name, shape, dram):
                t = wp.tile(shape, BF16, tag=name, name=name)
                nc.sync.dma_start(t[:], dram)
                return t

            wgi1 = wtile("wgi1", [DXA, 768], wgi1_d[:])
            wgh1rz = [wtile(f"wgh1rz{k}", [128, 512], wgh1rz_d[k]) for k in range(2)]
            wgh1n = [wtile(f"wgh1n{k}", [128, 256], wgh1n_d[k]) for k in range(2)]
            bhh1n = wtile("bhh1n", [2, 128], bhh1n_d[:])
            wgi2rz = [wtile(f"wgi2rz{k}", [128, 512], wgi2rz_d[k]) for k in range(2)]
            wgi2n = [wtile(f"wgi2n{k}", [128, 256], wgi2n_d[k]) for k in range(2)]
            wgh2rz = [wtile(f"wgh2rz{k}", [128, 512], wgh2rz_d[k]) for k in range(2)]
            wgh2n = [wtile(f"wgh2n{k}", [128, 256], wgh2n_d[k]) for k in range(2)]
            brz2 = wtile("brz2", [4, 128], brz2_d[:])
            bin2 = wtile("bin2", [2, 128], bin2_d[:])
            bhn2 = wtile("bhn2", [2, 128], bhn2_d[:])
            wlin = [wtile(f"wlin{k}", [128, REP], wlin_d[k]) for k in range(2)]
            blin = wtile("blin", [1, REP], blin_d[:])
            ident = wtile("ident", [128, 128], ident_d[:])
            pat2 = wtile("pat2", [2, 128], pat2_d[:])
            pat4 = wtile("pat4", [4, 256], pat4_d[:])
            ones = wtile("ones", [1, BL], ones_d[:])

            # ---- state: [128, 128] bf16, cols 0:64 = h dims 0:128,
            #      cols 64:128 = h dims 128:256 (per batch col) ----
            s1 = sp.tile([128, 128], BF16, tag="s1", name="s1_init")
            s2 = sp.tile([128, 128], BF16, tag="s2", name="s2_init")
            nc.vector.memset(s1[:].bitcast(F32), 0.0)
            nc.vector.memset(s2[:].bitcast(F32), 0.0)

            xchunks = [None] * NCHUNKS

            def load_chunk(c):
                xc = xp.tile([DXA, CHUNK, BL], BF16, tag="xchunk", name=f"xc{c}")
                nc.sync.dma_start(xc[:], xt_d[:, c * CHUNK:(c + 1) * CHUNK, :])
                return xc

            xchunks[0] = load_chunk(0)
            xchunks[1] = load_chunk(1)

            mm = nc.tensor.matmul

            for t in range(n_super):
                l1 = t < S
                l2 = t >= 1
                if l1:
                    c, j = divmod(t, CHUNK)
                    if j == 0 and c + 2 < NCHUNKS:
                        xchunks[c + 2] = load_chunk(c + 2)
                    xa = xchunks[c][:, j, :]   # [81, 64] moving

                # PSUM banks for this step (pool rotates parity)
                if l1:
                    rz1 = gp.tile([128, 512], F32, tag="rz1", name=f"rz1_{t}")
                    n1 = gp.tile([128, 512], F32, tag="n1", name=f"n1_{t}")
                if l2:
                    rz2 = gp.tile([128, 512], F32, tag="rz2", name=f"rz2_{t}")
                    n2 = gp.tile([128, 512], F32, tag="n2", name=f"n2_{t}")

                # start=True clears has_written for the WHOLE psum bank, so
                # each bank gets exactly one start=True (its first mm of the
                # step); later writes to untouched elements overwrite+set,
                # and accumulates (incl. the deferred v-accum) stay intact.
                def bank_mm(first):
                    st = {"v": first}
                    def f(o, lhsT, rhs, stop=False):
                        mm(o, lhsT, rhs, start=st["v"], stop=stop,
                           skip_group_check=True)
                        st["v"] = False
                    return f

                # ---- PE: L1 matmuls ----
                if l1:
                    b_rz1 = bank_mm(True)
                    for ch in range(4):   # r c0, r c1, z c0, z c1
                        o = rz1[:, 64 * ch:64 * (ch + 1)]
                        wcol = slice(128 * ch, 128 * (ch + 1))
                        b_rz1(o, wgi1[:, wcol], xa)
                        b_rz1(o, wgh1rz[0][:, wcol], s1[:, 0:64])
                        b_rz1(o, wgh1rz[1][:, wcol], s1[:, 64:128], stop=(ch == 3))
                    b_n1 = bank_mm(True)
                    for ch in range(2):   # i_n chunks (bank closed by v1-accum)
                        o = n1[:, 64 * ch:64 * (ch + 1)]
                        b_n1(o, wgi1[:, 512 + 128 * ch:512 + 128 * (ch + 1)], xa)
                    for ch in range(2):   # h_n chunks
                        o = n1[:, 128 + 64 * ch:128 + 64 * (ch + 1)]
                        wcol = slice(128 * ch, 128 * (ch + 1))
                        b_n1(o, wgh1n[0][:, wcol], s1[:, 0:64])
                        b_n1(o, wgh1n[1][:, wcol], s1[:, 64:128])
                    b_n1(n1[:, 128:256], bhh1n[:], pat2[:])

                # ---- PE: L2 matmuls (h1_{t-1} = s1, h2_{t-2} = s2) ----
                if l2:
                    b_rz2 = bank_mm(True)
                    for ch in range(4):
                        o = rz2[:, 64 * ch:64 * (ch + 1)]
                        wcol = slice(128 * ch, 128 * (ch + 1))
                        b_rz2(o, wgi2rz[0][:, wcol], s1[:, 0:64])
                        b_rz2(o, wgi2rz[1][:, wcol], s1[:, 64:128])
                        b_rz2(o, wgh2rz[0][:, wcol], s2[:, 0:64])
                        b_rz2(o, wgh2rz[1][:, wcol], s2[:, 64:128])
                    b_rz2(rz2[:, 0:256], brz2[:], pat4[:], stop=True)
                    b_n2 = bank_mm(True)
                    for ch in range(2):
                        o = n2[:, 64 * ch:64 * (ch + 1)]
                        wcol = slice(128 * ch, 128 * (ch + 1))
                        b_n2(o, wgi2n[0][:, wcol], s1[:, 0:64])
                        b_n2(o, wgi2n[1][:, wcol], s1[:, 64:128])
                    b_n2(n2[:, 0:128], bin2[:], pat2[:])
                    for ch in range(2):
                        o = n2[:, 128 + 64 * ch:128 + 64 * (ch + 1)]
                        wcol = slice(128 * ch, 128 * (ch + 1))
                        b_n2(o, wgh2n[0][:, wcol], s2[:, 0:64])
                        b_n2(o, wgh2n[1][:, wcol], s2[:, 64:128])
                    b_n2(n2[:, 128:256], bhn2[:], pat2[:])

                # ---- L1 chain ----
                if l1:
                    r1 = wk.tile([128, 128], BF16, tag="r1", name=f"r1_{t}")
                    w1 = wk.tile([128, 128], BF16, tag="w1", name=f"w1_{t}")
                    u1 = wk.tile([128, 128], BF16, tag="u1", name=f"u1_{t}")
                    n1s = wk.tile([128, 128], BF16, tag="n1s", name=f"n1s_{t}")
                    q1 = wk.tile([128, 128], BF16, tag="q1", name=f"q1_{t}")
                    p1 = wk.tile([128, 128], BF16, tag="p1", name=f"p1_{t}")
                    m1 = wk.tile([128, 128], BF16, tag="m1", name=f"m1_{t}")
                    s1n = sp.tile([128, 128], BF16, tag="s1", name=f"s1_{t}")
                    nc.scalar.activation(r1[:], rz1[:, 0:128], AF.Sigmoid)
                    nc.scalar.activation(w1[:], rz1[:, 128:256], AF.Sigmoid, scale=-1.0)
                    nc.vector.tensor_mul(u1[:], r1[:], n1[:, 128:256])
                    mm(n1[:, 0:128], ident[:], u1[:], start=False, stop=True, skip_group_check=True)  # v1
                    nc.scalar.activation(n1s[:], n1[:, 0:128], AF.Tanh)
                    nc.vector.tensor_mul(q1[:], w1[:], s1[:])
                    nc.vector.tensor_sub(p1[:], s1[:], q1[:])
                    nc.vector.tensor_mul(m1[:], w1[:], n1s[:])
                    nc.vector.tensor_add(s1n[:], m1[:], p1[:])

                # ---- L2 chain ----
                if l2:
                    r2 = wk.tile([128, 128], BF16, tag="r2", name=f"r2_{t}")
                    w2 = wk.tile([128, 128], BF16, tag="w2", name=f"w2_{t}")
                    u2 = wk.tile([128, 128], BF16, tag="u2", name=f"u2_{t}")
                    n2s = wk.tile([128, 128], BF16, tag="n2s", name=f"n2s_{t}")
                    q2 = wk.tile([128, 128], BF16, tag="q2", name=f"q2_{t}")
                    p2 = wk.tile([128, 128], BF16, tag="p2", name=f"p2_{t}")
                    m2 = wk.tile([128, 128], BF16, tag="m2", name=f"m2_{t}")
                    s2n = sp.tile([128, 128], BF16, tag="s2", name=f"s2_{t}")
                    nc.scalar.activation(r2[:], rz2[:, 0:128], AF.Sigmoid)
                    nc.scalar.activation(w2[:], rz2[:, 128:256], AF.Sigmoid, scale=-1.0)
                    nc.vector.tensor_mul(u2[:], r2[:], n2[:, 128:256])
                    mm(n2[:, 0:128], ident[:], u2[:], start=False, stop=True, skip_group_check=True)  # v2
                    nc.scalar.activation(n2s[:], n2[:, 0:128], AF.Tanh)
                    nc.vector.tensor_mul(q2[:], w2[:], s2[:])
                    nc.vector.tensor_sub(p2[:], s2[:], q2[:])
                    nc.vector.tensor_mul(m2[:], w2[:], n2s[:])
                    nc.vector.tensor_add(s2n[:], m2[:], p2[:])

                if l1:
                    s1 = s1n
                if l2:
                    s2 = s2n

            if dumps:
                dump_specs = [
                    ("d_rz1", rz1[:, 0:256], [128, 256]),
                    ("d_n1", n1[:, 0:256], [128, 256]),
                    ("d_r1", r1[:], [128, 128]),
                    ("d_w1", w1[:], [128, 128]),
                    ("d_u1", u1[:], [128, 128]),
                    ("d_n1s", n1s[:], [128, 128]),
                    ("d_s1", s1[:], [128, 128]),
                ]
                if n_super >= 2:
                    dump_specs += [
                        ("d_rz2", rz2[:, 0:256], [128, 256]),
                        ("d_n2", n2[:, 0:256], [128, 256]),
                        ("d_u2", u2[:], [128, 128]),
                        ("d_s2", s2[:], [128, 128]),
                    ]
                for dn, ap, shp in dump_specs:
                    dd = nc.dram_tensor(dn, shp, F32, kind="ExternalOutput")
                    db = wk.tile(shp, F32, tag=dn, name=dn)
                    nc.scalar.copy(db[:], ap)
                    nc.sync.dma_start(dd[:], db[:])

            # ---- final linear: outT = W_lin @ h2 + b_lin (gate-major) ----
            lin = gp.tile([128, 512], F32, tag="rz1", name="lin_ps")
            mm(lin[:, 0:64], wlin[0][:], s2[:, 0:64], start=True, stop=False)
            mm(lin[:, 0:64], wlin[1][:], s2[:, 64:128], start=False, stop=False)
            mm(lin[:, 0:64], blin[:], ones[:], start=False, stop=True)
            osb = wk.tile([REP, BL], F32, tag="osb", name="osb")
            nc.scalar.copy(osb[:], lin[:, 0:64])
            nc.sync.dma_start(out_d[:], osb[:])

    nc.compile()
    return nc


def prep_inputs(input, cond, W_ih1, W_hh1, b_ih1, b_hh1, W_ih2, W_hh2,
                b_ih2, b_hh2, W_lin, b_lin, n_super=NSUPER):
    f = np.float32
    bf = bfloat16
    x = np.concatenate([np.asarray(input, f), np.asarray(cond, f)], axis=-1)

    W_ih1 = np.asarray(W_ih1, f); W_hh1 = np.asarray(W_hh1, f)
    b_ih1 = np.asarray(b_ih1, f); b_hh1 = np.asarray(b_hh1, f)
    W_ih2 = np.asarray(W_ih2, f); W_hh2 = np.asarray(W_hh2, f)
    b_ih2 = np.asarray(b_ih2, f); b_hh2 = np.asarray(b_hh2, f)

    Wih1T = W_ih1.T  # [80, 768]
    Whh1T = W_hh1.T  # [256, 768]
    Wih2T = W_ih2.T
    Whh2T = W_hh2.T

    wgi1 = np.zeros((DXA, 768), f)
    wgi1[0:80] = Wih1T
    wgi1[80, 0:512] = (b_ih1 + b_hh1)[0:512]
    wgi1[80, 512:768] = b_ih1[512:768]

    pat2 = np.zeros((2, 128), f)
    pat2[0, 0:64] = 1.0
    pat2[1, 64:128] = 1.0
    pat4 = np.zeros((4, 256), f)
    for g in range(4):
        pat4[g, 64 * g:64 * (g + 1)] = 1.0

    shared = {
        "wgi1": wgi1.astype(bf),
        "wgh1rz": Whh1T[:, 0:512].reshape(2, 128, 512).astype(bf),
        "wgh1n": Whh1T[:, 512:768].reshape(2, 128, 256).astype(bf),
        "bhh1n": b_hh1[512:768].reshape(2, 128).astype(bf),
        "wgi2rz": Wih2T[:, 0:512].reshape(2, 128, 512).astype(bf),
        "wgi2n": Wih2T[:, 512:768].reshape(2, 128, 256).astype(bf),
        "wgh2rz": Whh2T[:, 0:512].reshape(2, 128, 512).astype(bf),
        "wgh2n": Whh2T[:, 512:768].reshape(2, 128, 256).astype(bf),
        "brz2": (b_ih2 + b_hh2)[0:512].reshape(4, 128).astype(bf),
        "bin2": b_ih2[512:768].reshape(2, 128).astype(bf),
        "bhn2": b_hh2[512:768].reshape(2, 128).astype(bf),
        "wlin": np.asarray(W_lin, f).T.reshape(2, 128, REP).astype(bf),
        "blin": np.asarray(b_lin, f).reshape(1, REP).astype(bf),
        "ident": np.eye(128, dtype=f).astype(bf),
        "pat2": pat2.astype(bf),
        "pat4": pat4.astype(bf),
        "ones": np.ones((1, BL), f).astype(bf),
    }

    in_maps = []
    for cidx in range(NCORES):
        xs = x[cidx * BL:(cidx + 1) * BL]            # [64, S, 80]
        xt = np.empty((DXA, S, BL), f)
        xt[0:80] = xs.transpose(2, 1, 0)
        xt[80] = 1.0
        m = dict(shared)
        m["xt"] = xt.astype(bf)
        in_maps.append(m)
    return in_maps


_program_cache = {}


def kernel(**inputs) -> np.ndarray:
    in_maps = prep_inputs(**inputs)
    if "nc" not in _program_cache:
        _program_cache["nc"] = build_program()
    nc = _program_cache["nc"]
    res = bass_utils.run_bass_kernel_spmd(nc, in_maps, core_ids=list(range(NCORES)))
    return np.concatenate(
        [np.asarray(r["outT"], np.float32).T for r in res.results], axis=0
    )


# revision 5
# speedup vs baseline: 1.0019x; 1.0019x over previous
"""Trainium2 Bass kernel for a 2-layer GRU encoder (nn_Encoder_28028956574172).

Gate-major redesign (v2): weights are the matmul STATIONARY operand and the
transposed hidden state [h, batch] is the 64-wide MOVING operand, so gates
land directly in [gate, batch] layout. This
  - eliminates all per-step PE transposes (state is produced in-layout),
  - runs every matmul in bf16 (1 cycle/row at any moving size),
  - puts activations/elementwise on full 128-partition tiles (halves ACT/DVE
    element counts vs batch-major),
  - shortens the per-step cross-engine dependency cycle.

Per-step structure (super-step t = L1 GRU step t + L2 GRU step t-1):
  Six single-buffered PSUM banks (r1, z1, n1, r2, z2, n2) so no two
  concurrently-read regions share a bank (Tile's bank-granular tracker
  would chain their readers through the ACT pipeline delay). n-banks:
  i_n at cols 0:128, h_n at 128:256. Exactly one start=True per bank per
  step (start clears has_written BANK-WIDE on TRN2).
  ACT: r = sigmoid(r-bank); w = sigmoid(-(z-bank)) = 1-z (scale=-1);
       n = tanh(v)
  v = i_n + r*h_n is formed by a PE identity-matmul that ACCUMULATES
  u = r*h_n (DVE product) into the i_n PSUM region (cheaper than a DVE
  add hop). Update uses h' = w*n + (h - w*h): q=w*h and p=h-q run OFF
  the critical chain (only m=w*n is on it), and the next step's r-bank
  matmuls consume m and p SEPARATELY (W(m+p) = Wm + Wp, p arrives
  early), removing the h'=m+p hop from the recurrence-critical cycle.

Biases: L1 ride a ones-row appended to the input (row 80); b_hh1n and all
L2 biases are injected by tiny K<=2 "pattern" matmuls into PSUM.
"""

import numpy as np
from ml_dtypes import bfloat16

import concourse.bacc as bacc
import concourse.bass as bass
import concourse.mybir as mybir
import concourse.tile as tile
from concourse import bass_utils

F32 = mybir.dt.float32
BF16 = mybir.dt.bfloat16

B, S, DIN, DC, H, REP = 512, 1024, 64, 16, 256, 128
NCORES = 8
BL = B // NCORES          # 64 batch per core
DXA = DIN + DC + 1        # 81: input+cond+ones row
CHUNK = 128               # timesteps per input DMA chunk
NCHUNKS = S // CHUNK      # 8
NSUPER = S + 1


def build_program(n_super=NSUPER, dumps=False):
    nc = bacc.Bacc(
        "TRN2",
        target_bir_lowering=False,
        debug=False,
        enable_asserts=False,
        num_devices=NCORES,
    )

    # ---- DRAM I/O (bf16 unless noted) ----
    xt_d = nc.dram_tensor("xt", [DXA, S, BL], BF16, kind="ExternalInput")
    wgi1_d = nc.dram_tensor("wgi1", [DXA, 768], BF16, kind="ExternalInput")
    wgh1rz_d = nc.dram_tensor("wgh1rz", [2, 128, 512], BF16, kind="ExternalInput")
    wgh1n_d = nc.dram_tensor("wgh1n", [2, 128, 256], BF16, kind="ExternalInput")
    bhh1n_d = nc.dram_tensor("bhh1n", [2, 128], BF16, kind="ExternalInput")
    wgi2rz_d = nc.dram_tensor("wgi2rz", [2, 128, 512], BF16, kind="ExternalInput")
    wgi2n_d = nc.dram_tensor("wgi2n", [2, 128, 256], BF16, kind="ExternalInput")
    wgh2rz_d = nc.dram_tensor("wgh2rz", [2, 128, 512], BF16, kind="ExternalInput")
    wgh2n_d = nc.dram_tensor("wgh2n", [2, 128, 256], BF16, kind="ExternalInput")
    br2_d = nc.dram_tensor("br2", [2, 128], BF16, kind="ExternalInput")
    bz2_d = nc.dram_tensor("bz2", [2, 128], BF16, kind="ExternalInput")
    bin2_d = nc.dram_tensor("bin2", [2, 128], BF16, kind="ExternalInput")
    bhn2_d = nc.dram_tensor("bhn2", [2, 128], BF16, kind="ExternalInput")
    wlin_d = nc.dram_tensor("wlin", [2, 128, REP], BF16, kind="ExternalInput")
    blin_d = nc.dram_tensor("blin", [1, REP], BF16, kind="ExternalInput")
    ident_d = nc.dram_tensor("ident", [128, 128], BF16, kind="ExternalInput")
    pat2_d = nc.dram_tensor("pat2", [2, 128], BF16, kind="ExternalInput")
    pat4_d = nc.dram_tensor("pat4", [4, 256], BF16, kind="ExternalInput")
    ones_d = nc.dram_tensor("ones", [1, BL], BF16, kind="ExternalInput")
    out_d = nc.dram_tensor("outT", [REP, BL], F32, kind="ExternalOutput")

    AF = mybir.ActivationFunctionType

    with tile.TileContext(nc) as tc:
        with (
            tc.tile_pool(name="wpool", bufs=1) as wp,
            tc.tile_pool(name="xpool", bufs=3) as xp,
            tc.tile_pool(name="state", bufs=2) as sp,
            tc.tile_pool(name="work", bufs=2) as wk,
            tc.tile_pool(name="psum", bufs=2, space=bass.MemorySpace.PSUM) as gp,
        ):
            # ---- x chunk 0 first: its big DMA gates step 0 and must not
            # queue behind 19 serialized weight DMAs ----
            xchunks = [None] * NCHUNKS

            def load_chunk(c):
                xc = xp.tile([DXA, CHUNK, BL], BF16, tag="xchunk", name=f"xc{c}")
                nc.sync.dma_start(xc[:], xt_d[:, c * CHUNK:(c + 1) * CHUNK, :])
                return xc

            xchunks[0] = load_chunk(0)

            # ---- resident weights ----
            def wtile(name, shape, dram):
                t = wp.tile(shape, BF16, tag=name, name=name)
                nc.sync.dma_start(t[:], dram)
                return t

            wgi1 = wtile("wgi1", [DXA, 768], wgi1_d[:])
            wgh1rz = [wtile(f"wgh1rz{k}", [128, 512], wgh1rz_d[k]) for k in range(2)]
            wgh1n = [wtile(f"wgh1n{k}", [128, 256], wgh1n_d[k]) for k in range(2)]
            bhh1n = wtile("bhh1n", [2, 128], bhh1n_d[:])
            wgi2rz = [wtile(f"wgi2rz{k}", [128, 512], wgi2rz_d[k]) for k in range(2)]
            wgi2n = [wtile(f"wgi2n{k}", [128, 256], wgi2n_d[k]) for k in range(2)]
            wgh2rz = [wtile(f"wgh2rz{k}", [128, 512], wgh2rz_d[k]) for k in range(2)]
            wgh2n = [wtile(f"wgh2n{k}", [128, 256], wgh2n_d[k]) for k in range(2)]
            br2 = wtile("br2", [2, 128], br2_d[:])
            bz2 = wtile("bz2", [2, 128], bz2_d[:])
            bin2 = wtile("bin2", [2, 128], bin2_d[:])
            bhn2 = wtile("bhn2", [2, 128], bhn2_d[:])
            wlin = [wtile(f"wlin{k}", [128, REP], wlin_d[k]) for k in range(2)]
            blin = wtile("blin", [1, REP], blin_d[:])
            ident = wtile("ident", [128, 128], ident_d[:])
            pat2 = wtile("pat2", [2, 128], pat2_d[:])
            pat4 = wtile("pat4", [4, 256], pat4_d[:])
            ones = wtile("ones", [1, BL], ones_d[:])

            # ---- state: [128, 128] bf16, cols 0:64 = h dims 0:128,
            #      cols 64:128 = h dims 128:256 (per batch col) ----
            s1 = sp.tile([128, 128], BF16, tag="s1", name="s1_init")
            s2 = sp.tile([128, 128], BF16, tag="s2", name="s2_init")
            nc.vector.memset(s1[:].bitcast(F32), 0.0)
            nc.vector.memset(s2[:].bitcast(F32), 0.0)

            xchunks[1] = load_chunk(1)

            # 6 single-buffered PSUM banks: r/z split so sigmoid(r) and
            # sigmoid(-z) never touch the same bank (Tile's bank-granular
            # tracking would otherwise chain them through the ACT pipeline
            # delay). Layout per bank: two 64-col chunks.
            r1b = gp.tile([128, 512], F32, tag="r1b", bufs=1, name="r1b")
            z1b = gp.tile([128, 512], F32, tag="z1b", bufs=1, name="z1b")
            n1b = gp.tile([128, 512], F32, tag="n1b", bufs=1, name="n1b")
            r2b = gp.tile([128, 512], F32, tag="r2b", bufs=1, name="r2b")
            z2b = gp.tile([128, 512], F32, tag="z2b", bufs=1, name="z2b")
            n2b = gp.tile([128, 512], F32, tag="n2b", bufs=1, name="n2b")

            mm = nc.tensor.matmul
            m1p = p1p = m2p = p2p = None

            for t in range(n_super):
                l1 = t < S
                l2 = t >= 1
                if l1:
                    c, j = divmod(t, CHUNK)
                    if j == 0 and c + 2 < NCHUNKS:
                        xchunks[c + 2] = load_chunk(c + 2)
                    xa = xchunks[c][:, j, :]   # [81, 64] moving

                # start=True clears has_written for the WHOLE psum bank, so
                # each bank gets exactly one start=True (its first mm of the
                # step); later writes to untouched elements overwrite+set,
                # and accumulates (incl. the deferred v-accum) stay intact.
                def bank_mm(first):
                    st = {"v": first}
                    def f(o, lhsT, rhs, stop=False):
                        mm(o, lhsT, rhs, start=st["v"], stop=stop,
                           skip_group_check=True)
                        st["v"] = False
                    return f

                # ---- PE: L1 matmuls ----
                if l1:
                    b_r1 = bank_mm(True)
                    b_z1 = bank_mm(True)
                    for ch in range(2):   # r chunks
                        o = r1b[:, 64 * ch:64 * (ch + 1)]
                        wcol = slice(128 * ch, 128 * (ch + 1))
                        b_r1(o, wgi1[:, wcol], xa)
                        if m1p is None:
                            b_r1(o, wgh1rz[0][:, wcol], s1[:, 0:64])
                            b_r1(o, wgh1rz[1][:, wcol], s1[:, 64:128], stop=(ch == 1))
                        else:
                            # W(m+p) split: p arrives early (off-chain), only
                            # the m-path is serial -> removes the s1=m+p hop
                            # from the r-gate critical cycle.
                            b_r1(o, wgh1rz[0][:, wcol], p1p[:, 0:64])
                            b_r1(o, wgh1rz[1][:, wcol], p1p[:, 64:128])
                            b_r1(o, wgh1rz[0][:, wcol], m1p[:, 0:64])
                            b_r1(o, wgh1rz[1][:, wcol], m1p[:, 64:128], stop=(ch == 1))
                    for ch in range(2):   # z chunks
                        o = z1b[:, 64 * ch:64 * (ch + 1)]
                        wcol = slice(256 + 128 * ch, 256 + 128 * (ch + 1))
                        b_z1(o, wgi1[:, wcol], xa)
                        b_z1(o, wgh1rz[0][:, wcol], s1[:, 0:64])
                        b_z1(o, wgh1rz[1][:, wcol], s1[:, 64:128], stop=(ch == 1))
                    b_n1 = bank_mm(True)
                    for ch in range(2):   # i_n chunks (bank closed by v1-accum)
                        o = n1b[:, 64 * ch:64 * (ch + 1)]
                        b_n1(o, wgi1[:, 512 + 128 * ch:512 + 128 * (ch + 1)], xa)
                    for ch in range(2):   # h_n chunks
                        o = n1b[:, 128 + 64 * ch:128 + 64 * (ch + 1)]
                        wcol = slice(128 * ch, 128 * (ch + 1))
                        b_n1(o, wgh1n[0][:, wcol], s1[:, 0:64])
                        b_n1(o, wgh1n[1][:, wcol], s1[:, 64:128])
                    b_n1(n1b[:, 128:256], bhh1n[:], pat2[:])

                # ---- PE: L2 matmuls (h1_{t-1} = s1, h2_{t-2} = s2) ----
                if l2:
                    b_r2 = bank_mm(True)
                    b_z2 = bank_mm(True)
                    for ch in range(2):
                        o = r2b[:, 64 * ch:64 * (ch + 1)]
                        wcol = slice(128 * ch, 128 * (ch + 1))
                        b_r2(o, wgi2rz[0][:, wcol], s1[:, 0:64])
                        b_r2(o, wgi2rz[1][:, wcol], s1[:, 64:128])
                        if m2p is None:
                            b_r2(o, wgh2rz[0][:, wcol], s2[:, 0:64])
                            b_r2(o, wgh2rz[1][:, wcol], s2[:, 64:128])
                        else:
                            b_r2(o, wgh2rz[0][:, wcol], p2p[:, 0:64])
                            b_r2(o, wgh2rz[1][:, wcol], p2p[:, 64:128])
                            b_r2(o, wgh2rz[0][:, wcol], m2p[:, 0:64])
                            b_r2(o, wgh2rz[1][:, wcol], m2p[:, 64:128])
                    b_r2(r2b[:, 0:128], br2[:], pat2[:], stop=True)
                    for ch in range(2):
                        o = z2b[:, 64 * ch:64 * (ch + 1)]
                        wcol = slice(256 + 128 * ch, 256 + 128 * (ch + 1))
                        b_z2(o, wgi2rz[0][:, wcol], s1[:, 0:64])
                        b_z2(o, wgi2rz[1][:, wcol], s1[:, 64:128])
                        b_z2(o, wgh2rz[0][:, wcol], s2[:, 0:64])
                        b_z2(o, wgh2rz[1][:, wcol], s2[:, 64:128])
                    b_z2(z2b[:, 0:128], bz2[:], pat2[:], stop=True)
                    b_n2 = bank_mm(True)
                    for ch in range(2):
                        o = n2b[:, 64 * ch:64 * (ch + 1)]
                        wcol = slice(128 * ch, 128 * (ch + 1))
                        b_n2(o, wgi2n[0][:, wcol], s1[:, 0:64])
                        b_n2(o, wgi2n[1][:, wcol], s1[:, 64:128])
                    b_n2(n2b[:, 0:128], bin2[:], pat2[:])
                    for ch in range(2):
                        o = n2b[:, 128 + 64 * ch:128 + 64 * (ch + 1)]
                        wcol = slice(128 * ch, 128 * (ch + 1))
                        b_n2(o, wgh2n[0][:, wcol], s2[:, 0:64])
                        b_n2(o, wgh2n[1][:, wcol], s2[:, 64:128])
                    b_n2(n2b[:, 128:256], bhn2[:], pat2[:])

                # ---- L1 chain ----
                if l1:
                    r1 = wk.tile([128, 128], BF16, tag="r1", name=f"r1_{t}")
                    w1 = wk.tile([128, 128], BF16, tag="w1", name=f"w1_{t}")
                    u1 = wk.tile([128, 128], BF16, tag="u1", name=f"u1_{t}")
                    n1s = wk.tile([128, 128], BF16, tag="n1s", name=f"n1s_{t}")
                    q1 = wk.tile([128, 128], BF16, tag="q1", name=f"q1_{t}")
                    p1 = wk.tile([128, 128], BF16, tag="p1", name=f"p1_{t}")
                    m1 = wk.tile([128, 128], BF16, tag="m1", name=f"m1_{t}")
                    s1n = sp.tile([128, 128], BF16, tag="s1", name=f"s1_{t}")
                    nc.scalar.activation(r1[:], r1b[:, 0:128], AF.Sigmoid)
                    nc.scalar.activation(w1[:], z1b[:, 0:128], AF.Sigmoid, scale=-1.0)
                    nc.vector.tensor_mul(u1[:], r1[:], n1b[:, 128:256])
                    mm(n1b[:, 0:128], ident[:], u1[:], start=False, stop=True, skip_group_check=True)  # v1
                    nc.scalar.activation(n1s[:], n1b[:, 0:128], AF.Tanh)
                    nc.vector.tensor_mul(q1[:], w1[:], s1[:])
                    nc.vector.tensor_sub(p1[:], s1[:], q1[:])
                    nc.vector.tensor_mul(m1[:], w1[:], n1s[:])
                    nc.vector.tensor_add(s1n[:], m1[:], p1[:])

                # ---- L2 chain ----
                if l2:
                    r2 = wk.tile([128, 128], BF16, tag="r2", name=f"r2_{t}")
                    w2 = wk.tile([128, 128], BF16, tag="w2", name=f"w2_{t}")
                    u2 = wk.tile([128, 128], BF16, tag="u2", name=f"u2_{t}")
                    n2s = wk.tile([128, 128], BF16, tag="n2s", name=f"n2s_{t}")
                    q2 = wk.tile([128, 128], BF16, tag="q2", name=f"q2_{t}")
                    p2 = wk.tile([128, 128], BF16, tag="p2", name=f"p2_{t}")
                    m2 = wk.tile([128, 128], BF16, tag="m2", name=f"m2_{t}")
                    s2n = sp.tile([128, 128], BF16, tag="s2", name=f"s2_{t}")
                    nc.scalar.activation(r2[:], r2b[:, 0:128], AF.Sigmoid)
                    nc.scalar.activation(w2[:], z2b[:, 0:128], AF.Sigmoid, scale=-1.0)
                    nc.vector.tensor_mul(u2[:], r2[:], n2b[:, 128:256])
                    mm(n2b[:, 0:128], ident[:], u2[:], start=False, stop=True, skip_group_check=True)  # v2
                    nc.scalar.activation(n2s[:], n2b[:, 0:128], AF.Tanh)
                    nc.vector.tensor_mul(q2[:], w2[:], s2[:])
                    nc.vector.tensor_sub(p2[:], s2[:], q2[:])
                    nc.vector.tensor_mul(m2[:], w2[:], n2s[:])
                    nc.vector.tensor_add(s2n[:], m2[:], p2[:])

                if l1:
                    s1 = s1n
                    m1p, p1p = m1, p1
                if l2:
                    s2 = s2n
                    m2p, p2p = m2, p2

            if dumps:
                dump_specs = [
                    ("d_rz1", r1b[:, 0:128], [128, 128]),
                    ("d_z1", z1b[:, 0:128], [128, 128]),
                    ("d_n1", n1b[:, 0:256], [128, 256]),
                    ("d_r1", r1[:], [128, 128]),
                    ("d_w1", w1[:], [128, 128]),
                    ("d_u1", u1[:], [128, 128]),
                    ("d_n1s", n1s[:], [128, 128]),
                    ("d_s1", s1[:], [128, 128]),
                ]
                if n_super >= 2:
                    dump_specs += [
                        ("d_rz2", r2b[:, 0:128], [128, 128]),
                        ("d_z2", z2b[:, 0:128], [128, 128]),
                        ("d_n2", n2b[:, 0:256], [128, 256]),
                        ("d_u2", u2[:], [128, 128]),
                        ("d_s2", s2[:], [128, 128]),
                    ]
                for dn, ap, shp in dump_specs:
                    dd = nc.dram_tensor(dn, shp, F32, kind="ExternalOutput")
                    db = wk.tile(shp, F32, tag=dn, name=dn)
                    nc.scalar.copy(db[:], ap)
                    nc.sync.dma_start(dd[:], db[:])

            # ---- final linear: outT = W_lin @ h2 + b_lin (gate-major) ----
            lin = r1b
            mm(lin[:, 0:64], wlin[0][:], s2[:, 0:64], start=True, stop=False, skip_group_check=True)
            mm(lin[:, 0:64], wlin[1][:], s2[:, 64:128], start=False, stop=False, skip_group_check=True)
            mm(lin[:, 0:64], blin[:], ones[:], start=False, stop=True, skip_group_check=True)
            osb = wk.tile([REP, BL], F32, tag="osb", name="osb")
            nc.scalar.copy(osb[:], lin[:, 0:64])
            nc.sync.dma_start(out_d[:], osb[:])

    nc.compile()
    return nc


def prep_inputs(input, cond, W_ih1, W_hh1, b_ih1, b_hh1, W_ih2, W_hh2,
                b_ih2, b_hh2, W_lin, b_lin, n_super=NSUPER):
    f = np.float32
    bf = bfloat16
    x = np.concatenate([np.asarray(input, f), np.asarray(cond, f)], axis=-1)

    W_ih1 = np.asarray(W_ih1, f); W_hh1 = np.asarray(W_hh1, f)
    b_ih1 = np.asarray(b_ih1, f); b_hh1 = np.asarray(b_hh1, f)
    W_ih2 = np.asarray(W_ih2, f); W_hh2 = np.asarray(W_hh2, f)
    b_ih2 = np.asarray(b_ih2, f); b_hh2 = np.asarray(b_hh2, f)

    Wih1T = W_ih1.T  # [80, 768]
    Whh1T = W_hh1.T  # [256, 768]
    Wih2T = W_ih2.T
    Whh2T = W_hh2.T

    wgi1 = np.zeros((DXA, 768), f)
    wgi1[0:80] = Wih1T
    wgi1[80, 0:512] = (b_ih1 + b_hh1)[0:512]
    wgi1[80, 512:768] = b_ih1[512:768]

    pat2 = np.zeros((2, 128), f)
    pat2[0, 0:64] = 1.0
    pat2[1, 64:128] = 1.0
    pat4 = np.zeros((4, 256), f)
    for g in range(4):
        pat4[g, 64 * g:64 * (g + 1)] = 1.0

    shared = {
        "wgi1": wgi1.astype(bf),
        "wgh1rz": Whh1T[:, 0:512].reshape(2, 128, 512).astype(bf),
        "wgh1n": Whh1T[:, 512:768].reshape(2, 128, 256).astype(bf),
        "bhh1n": b_hh1[512:768].reshape(2, 128).astype(bf),
        "wgi2rz": Wih2T[:, 0:512].reshape(2, 128, 512).astype(bf),
        "wgi2n": Wih2T[:, 512:768].reshape(2, 128, 256).astype(bf),
        "wgh2rz": Whh2T[:, 0:512].reshape(2, 128, 512).astype(bf),
        "wgh2n": Whh2T[:, 512:768].reshape(2, 128, 256).astype(bf),
        "br2": (b_ih2 + b_hh2)[0:256].reshape(2, 128).astype(bf),
        "bz2": (b_ih2 + b_hh2)[256:512].reshape(2, 128).astype(bf),
        "bin2": b_ih2[512:768].reshape(2, 128).astype(bf),
        "bhn2": b_hh2[512:768].reshape(2, 128).astype(bf),
        "wlin": np.asarray(W_lin, f).T.reshape(2, 128, REP).astype(bf),
        "blin": np.asarray(b_lin, f).reshape(1, REP).astype(bf),
        "ident": np.eye(128, dtype=f).astype(bf),
        "pat2": pat2.astype(bf),
        "pat4": pat4.astype(bf),
        "ones": np.ones((1, BL), f).astype(bf),
    }

    in_maps = []
    for cidx in range(NCORES):
        xs = x[cidx * BL:(cidx + 1) * BL]            # [64, S, 80]
        xt = np.empty((DXA, S, BL), f)
        xt[0:80] = xs.transpose(2, 1, 0)
        xt[80] = 1.0
        m = dict(shared)
        m["xt"] = xt.astype(bf)
        in_maps.append(m)
    return in_maps


_program_cache = {}


def kernel(**inputs) -> np.ndarray:
    in_maps = prep_inputs(**inputs)
    if "nc" not in _program_cache:
        _program_cache["nc"] = build_program()
    nc = _program_cache["nc"]
    res = bass_utils.run_bass_kernel_spmd(nc, in_maps, core_ids=list(range(NCORES)))
    return np.concatenate(
        [np.asarray(r["outT"], np.float32).T for r in res.results], axis=0
    )


# revision 6
# speedup vs baseline: 1.0019x; 1.0000x over previous
"""Trainium2 Bass kernel for a 2-layer GRU encoder (nn_Encoder_28028956574172).

Gate-major redesign (v2): weights are the matmul STATIONARY operand and the
transposed hidden state [h, batch] is the 64-wide MOVING operand, so gates
land directly in [gate, batch] layout. This
  - eliminates all per-step PE transposes (state is produced in-layout),
  - runs every matmul in bf16 (1 cycle/row at any moving size),
  - puts activations/elementwise on full 128-partition tiles (halves ACT/DVE
    element counts vs batch-major),
  - shortens the per-step cross-engine dependency cycle.

Per-step structure (super-step t = L1 GRU step t + L2 GRU step t-1):
  Six single-buffered PSUM banks (r1, z1, n1, r2, z2, n2) so no two
  concurrently-read regions share a bank (Tile's bank-granular tracker
  would chain their readers through the ACT pipeline delay). n-banks:
  i_n at cols 0:128, h_n at 128:256. Exactly one start=True per bank per
  step (start clears has_written BANK-WIDE on TRN2).
  ACT: r = sigmoid(r-bank); w = sigmoid(-(z-bank)) = 1-z (scale=-1);
       n = tanh(v)
  v = i_n + r*h_n is formed by a PE identity-matmul that ACCUMULATES
  u = r*h_n (DVE product) into the i_n PSUM region (cheaper than a DVE
  add hop). Update uses h' = w*n + (h - w*h): q=w*h and p=h-q run OFF
  the critical chain (only m=w*n is on it), and the next step's r-bank
  matmuls consume m and p SEPARATELY (W(m+p) = Wm + Wp, p arrives
  early), removing the h'=m+p hop from the recurrence-critical cycle.

Biases: L1 ride a ones-row appended to the input (row 80); b_hh1n and all
L2 biases are injected by tiny K<=2 "pattern" matmuls into PSUM.
"""

import numpy as np
from ml_dtypes import bfloat16

import concourse.bacc as bacc
import concourse.bass as bass
import concourse.mybir as mybir
import concourse.tile as tile
from concourse import bass_utils

F32 = mybir.dt.float32
BF16 = mybir.dt.bfloat16

B, S, DIN, DC, H, REP = 512, 1024, 64, 16, 256, 128
NCORES = 8
BL = B // NCORES          # 64 batch per core
DXA = DIN + DC + 1        # 81: input+cond+ones row
CHUNK = 128               # timesteps per input DMA chunk
NCHUNKS = S // CHUNK      # 8
NSUPER = S + 1


def build_program(n_super=NSUPER, dumps=False):
    nc = bacc.Bacc(
        "TRN2",
        target_bir_lowering=False,
        debug=False,
        enable_asserts=False,
        num_devices=NCORES,
    )

    # ---- DRAM I/O (bf16 unless noted) ----
    xt_d = nc.dram_tensor("xt", [DXA, S, BL], BF16, kind="ExternalInput")
    wgi1_d = nc.dram_tensor("wgi1", [DXA, 768], BF16, kind="ExternalInput")
    wgh1rz_d = nc.dram_tensor("wgh1rz", [2, 128, 512], BF16, kind="ExternalInput")
    wgh1n_d = nc.dram_tensor("wgh1n", [2, 128, 256], BF16, kind="ExternalInput")
    bhh1n_d = nc.dram_tensor("bhh1n", [2, 128], BF16, kind="ExternalInput")
    wgi2rz_d = nc.dram_tensor("wgi2rz", [2, 128, 512], BF16, kind="ExternalInput")
    wgi2n_d = nc.dram_tensor("wgi2n", [2, 128, 256], BF16, kind="ExternalInput")
    wgh2rz_d = nc.dram_tensor("wgh2rz", [2, 128, 512], BF16, kind="ExternalInput")
    wgh2n_d = nc.dram_tensor("wgh2n", [2, 128, 256], BF16, kind="ExternalInput")
    br2_d = nc.dram_tensor("br2", [2, 128], BF16, kind="ExternalInput")
    bz2_d = nc.dram_tensor("bz2", [2, 128], BF16, kind="ExternalInput")
    bin2_d = nc.dram_tensor("bin2", [2, 128], BF16, kind="ExternalInput")
    bhn2_d = nc.dram_tensor("bhn2", [2, 128], BF16, kind="ExternalInput")
    wlin_d = nc.dram_tensor("wlin", [2, 128, REP], BF16, kind="ExternalInput")
    blin_d = nc.dram_tensor("blin", [1, REP], BF16, kind="ExternalInput")
    ident_d = nc.dram_tensor("ident", [128, 128], BF16, kind="ExternalInput")
    pat2_d = nc.dram_tensor("pat2", [2, 128], BF16, kind="ExternalInput")
    ones_d = nc.dram_tensor("ones", [1, BL], BF16, kind="ExternalInput")
    out_d = nc.dram_tensor("outT", [REP, BL], F32, kind="ExternalOutput")

    AF = mybir.ActivationFunctionType

    with tile.TileContext(nc) as tc:
        with (
            tc.tile_pool(name="wpool", bufs=1) as wp,
            tc.tile_pool(name="xpool", bufs=3) as xp,
            tc.tile_pool(name="state", bufs=2) as sp,
            tc.tile_pool(name="work", bufs=2) as wk,
            tc.tile_pool(name="psum", bufs=2, space=bass.MemorySpace.PSUM) as gp,
        ):
            # ---- x chunk 0 first: its big DMA gates step 0 and must not
            # queue behind 19 serialized weight DMAs ----
            xchunks = [None] * NCHUNKS

            def load_chunk(c):
                xc = xp.tile([DXA, CHUNK, BL], BF16, tag="xchunk", name=f"xc{c}")
                nc.sync.dma_start(xc[:], xt_d[:, c * CHUNK:(c + 1) * CHUNK, :])
                return xc

            xchunks[0] = load_chunk(0)

            # ---- resident weights ----
            def wtile(name, shape, dram):
                t = wp.tile(shape, BF16, tag=name, name=name)
                nc.sync.dma_start(t[:], dram)
                return t

            wgi1 = wtile("wgi1", [DXA, 768], wgi1_d[:])
            wgh1rz = [wtile(f"wgh1rz{k}", [128, 512], wgh1rz_d[k]) for k in range(2)]
            wgh1n = [wtile(f"wgh1n{k}", [128, 256], wgh1n_d[k]) for k in range(2)]
            bhh1n = wtile("bhh1n", [2, 128], bhh1n_d[:])
            wgi2rz = [wtile(f"wgi2rz{k}", [128, 512], wgi2rz_d[k]) for k in range(2)]
            wgi2n = [wtile(f"wgi2n{k}", [128, 256], wgi2n_d[k]) for k in range(2)]
            wgh2rz = [wtile(f"wgh2rz{k}", [128, 512], wgh2rz_d[k]) for k in range(2)]
            wgh2n = [wtile(f"wgh2n{k}", [128, 256], wgh2n_d[k]) for k in range(2)]
            br2 = wtile("br2", [2, 128], br2_d[:])
            bz2 = wtile("bz2", [2, 128], bz2_d[:])
            bin2 = wtile("bin2", [2, 128], bin2_d[:])
            bhn2 = wtile("bhn2", [2, 128], bhn2_d[:])
            wlin = [wtile(f"wlin{k}", [128, REP], wlin_d[k]) for k in range(2)]
            blin = wtile("blin", [1, REP], blin_d[:])
            ident = wtile("ident", [128, 128], ident_d[:])
            pat2 = wtile("pat2", [2, 128], pat2_d[:])
            ones = wtile("ones", [1, BL], ones_d[:])

            # ---- state: [128, 128] bf16, cols 0:64 = h dims 0:128,
            #      cols 64:128 = h dims 128:256 (per batch col) ----
            s1 = sp.tile([128, 128], BF16, tag="s1", name="s1_init")
            s2 = sp.tile([128, 128], BF16, tag="s2", name="s2_init")
            nc.vector.memset(s1[:].bitcast(F32), 0.0)
            nc.vector.memset(s2[:].bitcast(F32), 0.0)

            xchunks[1] = load_chunk(1)

            # 6 single-buffered PSUM banks: r/z split so sigmoid(r) and
            # sigmoid(-z) never touch the same bank (Tile's bank-granular
            # tracking would otherwise chain them through the ACT pipeline
            # delay). Layout per bank: two 64-col chunks.
            r1b = gp.tile([128, 512], F32, tag="r1b", bufs=1, name="r1b")
            z1b = gp.tile([128, 512], F32, tag="z1b", bufs=1, name="z1b")
            n1b = gp.tile([128, 512], F32, tag="n1b", bufs=1, name="n1b")
            r2b = gp.tile([128, 512], F32, tag="r2b", bufs=1, name="r2b")
            z2b = gp.tile([128, 512], F32, tag="z2b", bufs=1, name="z2b")
            n2b = gp.tile([128, 512], F32, tag="n2b", bufs=1, name="n2b")

            mm = nc.tensor.matmul
            m1p = p1p = m2p = p2p = None

            for t in range(n_super):
                l1 = t < S
                l2 = t >= 1
                if l1:
                    c, j = divmod(t, CHUNK)
                    if j == 0 and c + 2 < NCHUNKS:
                        xchunks[c + 2] = load_chunk(c + 2)
                    xa = xchunks[c][:, j, :]   # [81, 64] moving

                # start=True clears has_written for the WHOLE psum bank, so
                # each bank gets exactly one start=True (its first mm of the
                # step); later writes to untouched elements overwrite+set,
                # and accumulates (incl. the deferred v-accum) stay intact.
                def bank_mm(first):
                    st = {"v": first}
                    def f(o, lhsT, rhs, stop=False):
                        mm(o, lhsT, rhs, start=st["v"], stop=stop,
                           skip_group_check=True)
                        st["v"] = False
                    return f

                # ---- PE: L1 matmuls ----
                if l1:
                    b_r1 = bank_mm(True)
                    b_z1 = bank_mm(True)
                    for ch in range(2):   # r chunks
                        o = r1b[:, 64 * ch:64 * (ch + 1)]
                        wcol = slice(128 * ch, 128 * (ch + 1))
                        b_r1(o, wgi1[:, wcol], xa)
                        if m1p is None:
                            b_r1(o, wgh1rz[0][:, wcol], s1[:, 0:64])
                            b_r1(o, wgh1rz[1][:, wcol], s1[:, 64:128], stop=(ch == 1))
                        else:
                            # W(m+p) split: p arrives early (off-chain), only
                            # the m-path is serial -> removes the s1=m+p hop
                            # from the r-gate critical cycle.
                            b_r1(o, wgh1rz[0][:, wcol], p1p[:, 0:64])
                            b_r1(o, wgh1rz[1][:, wcol], p1p[:, 64:128])
                            b_r1(o, wgh1rz[0][:, wcol], m1p[:, 0:64])
                            b_r1(o, wgh1rz[1][:, wcol], m1p[:, 64:128], stop=(ch == 1))
                    for ch in range(2):   # z chunks
                        o = z1b[:, 64 * ch:64 * (ch + 1)]
                        wcol = slice(256 + 128 * ch, 256 + 128 * (ch + 1))
                        b_z1(o, wgi1[:, wcol], xa)
                        b_z1(o, wgh1rz[0][:, wcol], s1[:, 0:64])
                        b_z1(o, wgh1rz[1][:, wcol], s1[:, 64:128], stop=(ch == 1))
                    b_n1 = bank_mm(True)
                    for ch in range(2):   # i_n chunks (bank closed by v1-accum)
                        o = n1b[:, 64 * ch:64 * (ch + 1)]
                        b_n1(o, wgi1[:, 512 + 128 * ch:512 + 128 * (ch + 1)], xa)
                    for ch in range(2):   # h_n chunks
                        o = n1b[:, 128 + 64 * ch:128 + 64 * (ch + 1)]
                        wcol = slice(128 * ch, 128 * (ch + 1))
                        b_n1(o, wgh1n[0][:, wcol], s1[:, 0:64])
                        b_n1(o, wgh1n[1][:, wcol], s1[:, 64:128])
                    b_n1(n1b[:, 128:256], bhh1n[:], pat2[:])

                # ---- PE: L2 matmuls (h1_{t-1} = s1, h2_{t-2} = s2) ----
                if l2:
                    b_r2 = bank_mm(True)
                    b_z2 = bank_mm(True)
                    for ch in range(2):
                        o = r2b[:, 64 * ch:64 * (ch + 1)]
                        wcol = slice(128 * ch, 128 * (ch + 1))
                        b_r2(o, wgi2rz[0][:, wcol], s1[:, 0:64])
                        b_r2(o, wgi2rz[1][:, wcol], s1[:, 64:128])
                        if m2p is None:
                            b_r2(o, wgh2rz[0][:, wcol], s2[:, 0:64])
                            b_r2(o, wgh2rz[1][:, wcol], s2[:, 64:128])
                        else:
                            b_r2(o, wgh2rz[0][:, wcol], p2p[:, 0:64])
                            b_r2(o, wgh2rz[1][:, wcol], p2p[:, 64:128])
                            b_r2(o, wgh2rz[0][:, wcol], m2p[:, 0:64])
                            b_r2(o, wgh2rz[1][:, wcol], m2p[:, 64:128])
                    b_r2(r2b[:, 0:128], br2[:], pat2[:], stop=True)
                    for ch in range(2):
                        o = z2b[:, 64 * ch:64 * (ch + 1)]
                        wcol = slice(256 + 128 * ch, 256 + 128 * (ch + 1))
                        b_z2(o, wgi2rz[0][:, wcol], s1[:, 0:64])
                        b_z2(o, wgi2rz[1][:, wcol], s1[:, 64:128])
                        b_z2(o, wgh2rz[0][:, wcol], s2[:, 0:64])
                        b_z2(o, wgh2rz[1][:, wcol], s2[:, 64:128])
                    b_z2(z2b[:, 0:128], bz2[:], pat2[:], stop=True)
                    b_n2 = bank_mm(True)
                    for ch in range(2):
                        o = n2b[:, 64 * ch:64 * (ch + 1)]
                        wcol = slice(128 * ch, 128 * (ch + 1))
                        b_n2(o, wgi2n[0][:, wcol], s1[:, 0:64])
                        b_n2(o, wgi2n[1][:, wcol], s1[:, 64:128])
                    b_n2(n2b[:, 0:128], bin2[:], pat2[:])
                    for ch in range(2):
                        o = n2b[:, 128 + 64 * ch:128 + 64 * (ch + 1)]
                        wcol = slice(128 * ch, 128 * (ch + 1))
                        b_n2(o, wgh2n[0][:, wcol], s2[:, 0:64])
                        b_n2(o, wgh2n[1][:, wcol], s2[:, 64:128])
                    b_n2(n2b[:, 128:256], bhn2[:], pat2[:])

                # ---- L1 chain ----
                if l1:
                    r1 = wk.tile([128, 128], BF16, tag="r1", name=f"r1_{t}")
                    w1 = wk.tile([128, 128], BF16, tag="w1", name=f"w1_{t}")
                    u1 = wk.tile([128, 128], BF16, tag="u1", name=f"u1_{t}")
                    n1s = wk.tile([128, 128], BF16, tag="n1s", name=f"n1s_{t}")
                    q1 = wk.tile([128, 128], BF16, tag="q1", name=f"q1_{t}")
                    p1 = wk.tile([128, 128], BF16, tag="p1", name=f"p1_{t}")
                    m1 = wk.tile([128, 128], BF16, tag="m1", name=f"m1_{t}")
                    s1n = sp.tile([128, 128], BF16, tag="s1", name=f"s1_{t}")
                    nc.scalar.activation(r1[:], r1b[:, 0:128], AF.Sigmoid)
                    nc.scalar.activation(w1[:], z1b[:, 0:128], AF.Sigmoid, scale=-1.0)
                    nc.vector.tensor_mul(u1[:], r1[:], n1b[:, 128:256])
                    mm(n1b[:, 0:128], ident[:], u1[:], start=False, stop=True, skip_group_check=True)  # v1
                    nc.scalar.activation(n1s[:], n1b[:, 0:128], AF.Tanh)
                    nc.vector.tensor_mul(q1[:], w1[:], s1[:])
                    nc.vector.tensor_sub(p1[:], s1[:], q1[:])
                    nc.vector.tensor_mul(m1[:], w1[:], n1s[:])
                    nc.vector.tensor_add(s1n[:], m1[:], p1[:])

                # ---- L2 chain ----
                if l2:
                    r2 = wk.tile([128, 128], BF16, tag="r2", name=f"r2_{t}")
                    w2 = wk.tile([128, 128], BF16, tag="w2", name=f"w2_{t}")
                    u2 = wk.tile([128, 128], BF16, tag="u2", name=f"u2_{t}")
                    n2s = wk.tile([128, 128], BF16, tag="n2s", name=f"n2s_{t}")
                    q2 = wk.tile([128, 128], BF16, tag="q2", name=f"q2_{t}")
                    p2 = wk.tile([128, 128], BF16, tag="p2", name=f"p2_{t}")
                    m2 = wk.tile([128, 128], BF16, tag="m2", name=f"m2_{t}")
                    s2n = sp.tile([128, 128], BF16, tag="s2", name=f"s2_{t}")
                    nc.scalar.activation(r2[:], r2b[:, 0:128], AF.Sigmoid)
                    nc.scalar.activation(w2[:], z2b[:, 0:128], AF.Sigmoid, scale=-1.0)
                    nc.vector.tensor_mul(u2[:], r2[:], n2b[:, 128:256])
                    mm(n2b[:, 0:128], ident[:], u2[:], start=False, stop=True, skip_group_check=True)  # v2
                    nc.scalar.activation(n2s[:], n2b[:, 0:128], AF.Tanh)
                    nc.vector.tensor_mul(q2[:], w2[:], s2[:])
                    nc.vector.tensor_sub(p2[:], s2[:], q2[:])
                    nc.vector.tensor_mul(m2[:], w2[:], n2s[:])
                    nc.vector.tensor_add(s2n[:], m2[:], p2[:])

                if l1:
                    s1 = s1n
                    m1p, p1p = m1, p1
                if l2:
                    s2 = s2n
                    m2p, p2p = m2, p2

            if dumps:
                dump_specs = [
                    ("d_rz1", r1b[:, 0:128], [128, 128]),
                    ("d_z1", z1b[:, 0:128], [128, 128]),
                    ("d_n1", n1b[:, 0:256], [128, 256]),
                    ("d_r1", r1[:], [128, 128]),
                    ("d_w1", w1[:], [128, 128]),
                    ("d_u1", u1[:], [128, 128]),
                    ("d_n1s", n1s[:], [128, 128]),
                    ("d_s1", s1[:], [128, 128]),
                ]
                if n_super >= 2:
                    dump_specs += [
                        ("d_rz2", r2b[:, 0:128], [128, 128]),
                        ("d_z2", z2b[:, 0:128], [128, 128]),
                        ("d_n2", n2b[:, 0:256], [128, 256]),
                        ("d_u2", u2[:], [128, 128]),
                        ("d_s2", s2[:], [128, 128]),
                    ]
                for dn, ap, shp in dump_specs:
                    dd = nc.dram_tensor(dn, shp, F32, kind="ExternalOutput")
                    db = wk.tile(shp, F32, tag=dn, name=dn)
                    nc.scalar.copy(db[:], ap)
                    nc.sync.dma_start(dd[:], db[:])

            # ---- final linear: outT = W_lin @ h2 + b_lin (gate-major) ----
            lin = r1b
            mm(lin[:, 0:64], wlin[0][:], s2[:, 0:64], start=True, stop=False, skip_group_check=True)
            mm(lin[:, 0:64], wlin[1][:], s2[:, 64:128], start=False, stop=False, skip_group_check=True)
            mm(lin[:, 0:64], blin[:], ones[:], start=False, stop=True, skip_group_check=True)
            osb = wk.tile([REP, BL], F32, tag="osb", name="osb")
            nc.scalar.copy(osb[:], lin[:, 0:64])
            nc.sync.dma_start(out_d[:], osb[:])

    nc.compile()
    return nc


def prep_inputs(input, cond, W_ih1, W_hh1, b_ih1, b_hh1, W_ih2, W_hh2,
                b_ih2, b_hh2, W_lin, b_lin, n_super=NSUPER):
    f = np.float32
    bf = bfloat16
    x = np.concatenate([np.asarray(input, f), np.asarray(cond, f)], axis=-1)

    W_ih1 = np.asarray(W_ih1, f); W_hh1 = np.asarray(W_hh1, f)
    b_ih1 = np.asarray(b_ih1, f); b_hh1 = np.asarray(b_hh1, f)
    W_ih2 = np.asarray(W_ih2, f); W_hh2 = np.asarray(W_hh2, f)
    b_ih2 = np.asarray(b_ih2, f); b_hh2 = np.asarray(b_hh2, f)

    Wih1T = W_ih1.T  # [80, 768]
    Whh1T = W_hh1.T  # [256, 768]
    Wih2T = W_ih2.T
    Whh2T = W_hh2.T

    wgi1 = np.zeros((DXA, 768), f)
    wgi1[0:80] = Wih1T
    wgi1[80, 0:512] = (b_ih1 + b_hh1)[0:512]
    wgi1[80, 512:768] = b_ih1[512:768]

    pat2 = np.zeros((2, 128), f)
    pat2[0, 0:64] = 1.0
    pat2[1, 64:128] = 1.0
    shared = {
        "wgi1": wgi1.astype(bf),
        "wgh1rz": Whh1T[:, 0:512].reshape(2, 128, 512).astype(bf),
        "wgh1n": Whh1T[:, 512:768].reshape(2, 128, 256).astype(bf),
        "bhh1n": b_hh1[512:768].reshape(2, 128).astype(bf),
        "wgi2rz": Wih2T[:, 0:512].reshape(2, 128, 512).astype(bf),
        "wgi2n": Wih2T[:, 512:768].reshape(2, 128, 256).astype(bf),
        "wgh2rz": Whh2T[:, 0:512].reshape(2, 128, 512).astype(bf),
        "wgh2n": Whh2T[:, 512:768].reshape(2, 128, 256).astype(bf),
        "br2": (b_ih2 + b_hh2)[0:256].reshape(2, 128).astype(bf),
        "bz2": (b_ih2 + b_hh2)[256:512].reshape(2, 128).astype(bf),
        "bin2": b_ih2[512:768].reshape(2, 128).astype(bf),
        "bhn2": b_hh2[512:768].reshape(2, 128).astype(bf),
        "wlin": np.asarray(W_lin, f).T.reshape(2, 128, REP).astype(bf),
        "blin": np.asarray(b_lin, f).reshape(1, REP).astype(bf),
        "ident": np.eye(128, dtype=f).astype(bf),
        "pat2": pat2.astype(bf),
        "ones": np.ones((1, BL), f).astype(bf),
    }

    in_maps = []
    for cidx in range(NCORES):
        xs = x[cidx * BL:(cidx + 1) * BL]            # [64, S, 80]
        xt = np.empty((DXA, S, BL), f)
        xt[0:80] = xs.transpose(2, 1, 0)
        xt[80] = 1.0
        m = dict(shared)
        m["xt"] = xt.astype(bf)
        in_maps.append(m)
    return in_maps


_program_cache = {}


def kernel(**inputs) -> np.ndarray:
    in_maps = prep_inputs(**inputs)
    if "nc" not in _program_cache:
        _program_cache["nc"] = build_program()
    nc = _program_cache["nc"]
    res = bass_utils.run_bass_kernel_spmd(nc, in_maps, core_ids=list(range(NCORES)))
    return np.concatenate(
        [np.asarray(r["outT"], np.float32).T for r in res.results], axis=0
    )


# revision 7
# speedup vs baseline: 1.0043x; 1.0024x over previous
"""Trainium2 Bass kernel for a 2-layer GRU encoder (nn_Encoder_28028956574172).

Gate-major redesign (v2): weights are the matmul STATIONARY operand and the
transposed hidden state [h, batch] is the 64-wide MOVING operand, so gates
land directly in [gate, batch] layout. This
  - eliminates all per-step PE transposes (state is produced in-layout),
  - runs every matmul in bf16 (1 cycle/row at any moving size),
  - puts activations/elementwise on full 128-partition tiles (halves ACT/DVE
    element counts vs batch-major),
  - shortens the per-step cross-engine dependency cycle.

Per-step structure (super-step t = L1 GRU step t + L2 GRU step t-1):
  Six single-buffered PSUM banks (r1, z1, n1, r2, z2, n2) so no two
  concurrently-read regions share a bank (Tile's bank-granular tracker
  would chain their readers through the ACT pipeline delay). n-banks:
  i_n at cols 0:128, h_n at 128:256. Exactly one start=True per bank per
  step (start clears has_written BANK-WIDE on TRN2).
  ACT: r = sigmoid(r-bank); w = sigmoid(-(z-bank)) = 1-z (scale=-1);
       n = tanh(v)
  v = i_n + r*h_n is formed by a PE identity-matmul that ACCUMULATES
  u = r*h_n (DVE product) into the i_n PSUM region (cheaper than a DVE
  add hop). Update uses h' = w*n + (h - w*h): q=w*h and p=h-q run OFF
  the critical chain (only m=w*n is on it), and the next step's r-bank
  matmuls consume m and p SEPARATELY (W(m+p) = Wm + Wp, p arrives
  early), removing the h'=m+p hop from the recurrence-critical cycle.

Biases: L1 ride a ones-row appended to the input (row 80); b_hh1n and all
L2 biases are injected by tiny K<=2 "pattern" matmuls into PSUM.
"""

import numpy as np
from ml_dtypes import bfloat16

import concourse.bacc as bacc
import concourse.bass as bass
import concourse.mybir as mybir
import concourse.tile as tile
from concourse import bass_utils

F32 = mybir.dt.float32
BF16 = mybir.dt.bfloat16

B, S, DIN, DC, H, REP = 512, 1024, 64, 16, 256, 128
NCORES = 8
BL = B // NCORES          # 64 batch per core
DXA = DIN + DC + 1        # 81: input+cond+ones row
CHUNK = 128               # timesteps per input DMA chunk
NCHUNKS = S // CHUNK      # 8
NSUPER = S + 1


def build_program(n_super=NSUPER, dumps=False):
    nc = bacc.Bacc(
        "TRN2",
        target_bir_lowering=False,
        debug=False,
        enable_asserts=False,
        num_devices=NCORES,
    )

    # ---- DRAM I/O (bf16 unless noted) ----
    xt_d = nc.dram_tensor("xt", [DXA, S, BL], BF16, kind="ExternalInput")
    wgi1_d = nc.dram_tensor("wgi1", [DXA, 768], BF16, kind="ExternalInput")
    wgh1rz_d = nc.dram_tensor("wgh1rz", [2, 128, 512], BF16, kind="ExternalInput")
    wgh1n_d = nc.dram_tensor("wgh1n", [2, 128, 256], BF16, kind="ExternalInput")
    bhh1n_d = nc.dram_tensor("bhh1n", [2, 128], BF16, kind="ExternalInput")
    wgi2rz_d = nc.dram_tensor("wgi2rz", [2, 128, 512], BF16, kind="ExternalInput")
    wgi2n_d = nc.dram_tensor("wgi2n", [2, 128, 256], BF16, kind="ExternalInput")
    wgh2rz_d = nc.dram_tensor("wgh2rz", [2, 128, 512], BF16, kind="ExternalInput")
    wgh2n_d = nc.dram_tensor("wgh2n", [2, 128, 256], BF16, kind="ExternalInput")
    br2_d = nc.dram_tensor("br2", [2, 128], BF16, kind="ExternalInput")
    bz2_d = nc.dram_tensor("bz2", [2, 128], BF16, kind="ExternalInput")
    bin2_d = nc.dram_tensor("bin2", [2, 128], BF16, kind="ExternalInput")
    bhn2_d = nc.dram_tensor("bhn2", [2, 128], BF16, kind="ExternalInput")
    wlin_d = nc.dram_tensor("wlin", [2, 128, REP], BF16, kind="ExternalInput")
    blin_d = nc.dram_tensor("blin", [1, REP], BF16, kind="ExternalInput")
    ident_d = nc.dram_tensor("ident", [128, 128], BF16, kind="ExternalInput")
    pat2_d = nc.dram_tensor("pat2", [2, 128], BF16, kind="ExternalInput")
    ones_d = nc.dram_tensor("ones", [1, BL], BF16, kind="ExternalInput")
    out_d = nc.dram_tensor("outT", [REP, BL], F32, kind="ExternalOutput")

    AF = mybir.ActivationFunctionType

    with tile.TileContext(nc) as tc:
        with (
            tc.tile_pool(name="wpool", bufs=1) as wp,
            tc.tile_pool(name="xpool", bufs=3) as xp,
            tc.tile_pool(name="state", bufs=2) as sp,
            tc.tile_pool(name="work", bufs=2) as wk,
            tc.tile_pool(name="psum", bufs=2, space=bass.MemorySpace.PSUM) as gp,
        ):
            # ---- x chunk 0 first: its big DMA gates step 0 and must not
            # queue behind 19 serialized weight DMAs ----
            xchunks = [None] * NCHUNKS

            def load_chunk(c):
                xc = xp.tile([DXA, CHUNK, BL], BF16, tag="xchunk", name=f"xc{c}")
                nc.sync.dma_start(xc[:], xt_d[:, c * CHUNK:(c + 1) * CHUNK, :])
                return xc

            xchunks[0] = load_chunk(0)

            # ---- resident weights; spread DMA issue across three idle
            # two sequencers (SP + ACT) so the ~600ns DGE setups parallelize ----
            def wtile(name, shape, dram, eng=None):
                t = wp.tile(shape, BF16, tag=name, name=name)
                (eng or nc.sync).dma_start(t[:], dram)
                return t

            wgi1 = wtile("wgi1", [DXA, 768], wgi1_d[:])
            wgh1rz = [wtile(f"wgh1rz{k}", [128, 512], wgh1rz_d[k]) for k in range(2)]
            wgh1n = [wtile(f"wgh1n{k}", [128, 256], wgh1n_d[k]) for k in range(2)]
            bhh1n = wtile("bhh1n", [2, 128], bhh1n_d[:])
            pat2 = wtile("pat2", [2, 128], pat2_d[:])
            ident = wtile("ident", [128, 128], ident_d[:])
            wgi2rz = [wtile(f"wgi2rz{k}", [128, 512], wgi2rz_d[k], nc.scalar) for k in range(2)]
            wgi2n = [wtile(f"wgi2n{k}", [128, 256], wgi2n_d[k], nc.scalar) for k in range(2)]
            wgh2rz = [wtile(f"wgh2rz{k}", [128, 512], wgh2rz_d[k], nc.scalar) for k in range(2)]
            wgh2n = [wtile(f"wgh2n{k}", [128, 256], wgh2n_d[k], nc.scalar) for k in range(2)]
            br2 = wtile("br2", [2, 128], br2_d[:])
            bz2 = wtile("bz2", [2, 128], bz2_d[:])
            bin2 = wtile("bin2", [2, 128], bin2_d[:])
            bhn2 = wtile("bhn2", [2, 128], bhn2_d[:])
            wlin = [wtile(f"wlin{k}", [128, REP], wlin_d[k]) for k in range(2)]
            blin = wtile("blin", [1, REP], blin_d[:])
            ones = wtile("ones", [1, BL], ones_d[:])

            # ---- state: [128, 128] bf16, cols 0:64 = h dims 0:128,
            #      cols 64:128 = h dims 128:256 (per batch col) ----
            s1 = sp.tile([128, 128], BF16, tag="s1", name="s1_init")
            s2 = sp.tile([128, 128], BF16, tag="s2", name="s2_init")
            nc.vector.memset(s1[:].bitcast(F32), 0.0)
            nc.vector.memset(s2[:].bitcast(F32), 0.0)

            xchunks[1] = load_chunk(1)

            # 6 single-buffered PSUM banks: r/z split so sigmoid(r) and
            # sigmoid(-z) never touch the same bank (Tile's bank-granular
            # tracking would otherwise chain them through the ACT pipeline
            # delay). Layout per bank: two 64-col chunks.
            r1b = gp.tile([128, 512], F32, tag="r1b", bufs=1, name="r1b")
            z1b = gp.tile([128, 512], F32, tag="z1b", bufs=1, name="z1b")
            n1b = gp.tile([128, 512], F32, tag="n1b", bufs=1, name="n1b")
            r2b = gp.tile([128, 512], F32, tag="r2b", bufs=1, name="r2b")
            z2b = gp.tile([128, 512], F32, tag="z2b", bufs=1, name="z2b")
            n2b = gp.tile([128, 512], F32, tag="n2b", bufs=1, name="n2b")

            mm = nc.tensor.matmul
            m1p = p1p = m2p = p2p = None

            for t in range(n_super):
                l1 = t < S
                l2 = t >= 1
                if l1:
                    c, j = divmod(t, CHUNK)
                    if j == 0 and c + 2 < NCHUNKS:
                        xchunks[c + 2] = load_chunk(c + 2)
                    xa = xchunks[c][:, j, :]   # [81, 64] moving

                # start=True clears has_written for the WHOLE psum bank, so
                # each bank gets exactly one start=True (its first mm of the
                # step); later writes to untouched elements overwrite+set,
                # and accumulates (incl. the deferred v-accum) stay intact.
                def bank_mm(first):
                    st = {"v": first}
                    def f(o, lhsT, rhs, stop=False):
                        mm(o, lhsT, rhs, start=st["v"], stop=stop,
                           skip_group_check=True)
                        st["v"] = False
                    return f

                # ---- PE: L1 matmuls ----
                if l1:
                    b_r1 = bank_mm(True)
                    b_z1 = bank_mm(True)
                    for ch in range(2):   # r chunks
                        o = r1b[:, 64 * ch:64 * (ch + 1)]
                        wcol = slice(128 * ch, 128 * (ch + 1))
                        b_r1(o, wgi1[:, wcol], xa)
                        if m1p is None:
                            b_r1(o, wgh1rz[0][:, wcol], s1[:, 0:64])
                            b_r1(o, wgh1rz[1][:, wcol], s1[:, 64:128], stop=(ch == 1))
                        else:
                            # W(m+p) split: p arrives early (off-chain), only
                            # the m-path is serial -> removes the s1=m+p hop
                            # from the r-gate critical cycle.
                            b_r1(o, wgh1rz[0][:, wcol], p1p[:, 0:64])
                            b_r1(o, wgh1rz[1][:, wcol], p1p[:, 64:128])
                            b_r1(o, wgh1rz[0][:, wcol], m1p[:, 0:64])
                            b_r1(o, wgh1rz[1][:, wcol], m1p[:, 64:128], stop=(ch == 1))
                    for ch in range(2):   # z chunks
                        o = z1b[:, 64 * ch:64 * (ch + 1)]
                        wcol = slice(256 + 128 * ch, 256 + 128 * (ch + 1))
                        b_z1(o, wgi1[:, wcol], xa)
                        b_z1(o, wgh1rz[0][:, wcol], s1[:, 0:64])
                        b_z1(o, wgh1rz[1][:, wcol], s1[:, 64:128], stop=(ch == 1))
                    b_n1 = bank_mm(True)
                    for ch in range(2):   # i_n chunks (bank closed by v1-accum)
                        o = n1b[:, 64 * ch:64 * (ch + 1)]
                        b_n1(o, wgi1[:, 512 + 128 * ch:512 + 128 * (ch + 1)], xa)
                    for ch in range(2):   # h_n chunks
                        o = n1b[:, 128 + 64 * ch:128 + 64 * (ch + 1)]
                        wcol = slice(128 * ch, 128 * (ch + 1))
                        b_n1(o, wgh1n[0][:, wcol], s1[:, 0:64])
                        b_n1(o, wgh1n[1][:, wcol], s1[:, 64:128])
                    b_n1(n1b[:, 128:256], bhh1n[:], pat2[:])

                # ---- PE: L2 matmuls (h1_{t-1} = s1, h2_{t-2} = s2) ----
                if l2:
                    b_r2 = bank_mm(True)
                    b_z2 = bank_mm(True)
                    for ch in range(2):
                        o = r2b[:, 64 * ch:64 * (ch + 1)]
                        wcol = slice(128 * ch, 128 * (ch + 1))
                        b_r2(o, wgi2rz[0][:, wcol], s1[:, 0:64])
                        b_r2(o, wgi2rz[1][:, wcol], s1[:, 64:128])
                        if m2p is None:
                            b_r2(o, wgh2rz[0][:, wcol], s2[:, 0:64])
                            b_r2(o, wgh2rz[1][:, wcol], s2[:, 64:128])
                        else:
                            b_r2(o, wgh2rz[0][:, wcol], p2p[:, 0:64])
                            b_r2(o, wgh2rz[1][:, wcol], p2p[:, 64:128])
                            b_r2(o, wgh2rz[0][:, wcol], m2p[:, 0:64])
                            b_r2(o, wgh2rz[1][:, wcol], m2p[:, 64:128])
                    b_r2(r2b[:, 0:128], br2[:], pat2[:], stop=True)
                    for ch in range(2):
                        o = z2b[:, 64 * ch:64 * (ch + 1)]
                        wcol = slice(256 + 128 * ch, 256 + 128 * (ch + 1))
                        b_z2(o, wgi2rz[0][:, wcol], s1[:, 0:64])
                        b_z2(o, wgi2rz[1][:, wcol], s1[:, 64:128])
                        b_z2(o, wgh2rz[0][:, wcol], s2[:, 0:64])
                        b_z2(o, wgh2rz[1][:, wcol], s2[:, 64:128])
                    b_z2(z2b[:, 0:128], bz2[:], pat2[:], stop=True)
                    b_n2 = bank_mm(True)
                    for ch in range(2):
                        o = n2b[:, 64 * ch:64 * (ch + 1)]
                        wcol = slice(128 * ch, 128 * (ch + 1))
                        b_n2(o, wgi2n[0][:, wcol], s1[:, 0:64])
                        b_n2(o, wgi2n[1][:, wcol], s1[:, 64:128])
                    b_n2(n2b[:, 0:128], bin2[:], pat2[:])
                    for ch in range(2):
                        o = n2b[:, 128 + 64 * ch:128 + 64 * (ch + 1)]
                        wcol = slice(128 * ch, 128 * (ch + 1))
                        b_n2(o, wgh2n[0][:, wcol], s2[:, 0:64])
                        b_n2(o, wgh2n[1][:, wcol], s2[:, 64:128])
                    b_n2(n2b[:, 128:256], bhn2[:], pat2[:])

                # ---- L1 chain ----
                if l1:
                    r1 = wk.tile([128, 128], BF16, tag="r1", name=f"r1_{t}")
                    w1 = wk.tile([128, 128], BF16, tag="w1", name=f"w1_{t}")
                    u1 = wk.tile([128, 128], BF16, tag="u1", name=f"u1_{t}")
                    n1s = wk.tile([128, 128], BF16, tag="n1s", name=f"n1s_{t}")
                    q1 = wk.tile([128, 128], BF16, tag="q1", name=f"q1_{t}")
                    p1 = wk.tile([128, 128], BF16, tag="p1", name=f"p1_{t}")
                    m1 = wk.tile([128, 128], BF16, tag="m1", name=f"m1_{t}")
                    s1n = sp.tile([128, 128], BF16, tag="s1", name=f"s1_{t}")
                    nc.scalar.activation(r1[:], r1b[:, 0:128], AF.Sigmoid)
                    nc.scalar.activation(w1[:], z1b[:, 0:128], AF.Sigmoid, scale=-1.0)
                    nc.vector.tensor_mul(u1[:], r1[:], n1b[:, 128:256])
                    mm(n1b[:, 0:128], ident[:], u1[:], start=False, stop=True, skip_group_check=True)  # v1
                    nc.scalar.activation(n1s[:], n1b[:, 0:128], AF.Tanh)
                    nc.vector.tensor_mul(q1[:], w1[:], s1[:])
                    nc.vector.tensor_sub(p1[:], s1[:], q1[:])
                    nc.vector.tensor_mul(m1[:], w1[:], n1s[:])
                    nc.vector.tensor_add(s1n[:], m1[:], p1[:])

                # ---- L2 chain ----
                if l2:
                    r2 = wk.tile([128, 128], BF16, tag="r2", name=f"r2_{t}")
                    w2 = wk.tile([128, 128], BF16, tag="w2", name=f"w2_{t}")
                    u2 = wk.tile([128, 128], BF16, tag="u2", name=f"u2_{t}")
                    n2s = wk.tile([128, 128], BF16, tag="n2s", name=f"n2s_{t}")
                    q2 = wk.tile([128, 128], BF16, tag="q2", name=f"q2_{t}")
                    p2 = wk.tile([128, 128], BF16, tag="p2", name=f"p2_{t}")
                    m2 = wk.tile([128, 128], BF16, tag="m2", name=f"m2_{t}")
                    s2n = sp.tile([128, 128], BF16, tag="s2", name=f"s2_{t}")
                    nc.scalar.activation(r2[:], r2b[:, 0:128], AF.Sigmoid)
                    nc.scalar.activation(w2[:], z2b[:, 0:128], AF.Sigmoid, scale=-1.0)
                    nc.vector.tensor_mul(u2[:], r2[:], n2b[:, 128:256])
                    mm(n2b[:, 0:128], ident[:], u2[:], start=False, stop=True, skip_group_check=True)  # v2
                    nc.scalar.activation(n2s[:], n2b[:, 0:128], AF.Tanh)
                    nc.vector.tensor_mul(q2[:], w2[:], s2[:])
                    nc.vector.tensor_sub(p2[:], s2[:], q2[:])
                    nc.vector.tensor_mul(m2[:], w2[:], n2s[:])
                    nc.vector.tensor_add(s2n[:], m2[:], p2[:])

                if l1:
                    s1 = s1n
                    m1p, p1p = m1, p1
                if l2:
                    s2 = s2n
                    m2p, p2p = m2, p2

            if dumps:
                dump_specs = [
                    ("d_rz1", r1b[:, 0:128], [128, 128]),
                    ("d_z1", z1b[:, 0:128], [128, 128]),
                    ("d_n1", n1b[:, 0:256], [128, 256]),
                    ("d_r1", r1[:], [128, 128]),
                    ("d_w1", w1[:], [128, 128]),
                    ("d_u1", u1[:], [128, 128]),
                    ("d_n1s", n1s[:], [128, 128]),
                    ("d_s1", s1[:], [128, 128]),
                ]
                if n_super >= 2:
                    dump_specs += [
                        ("d_rz2", r2b[:, 0:128], [128, 128]),
                        ("d_z2", z2b[:, 0:128], [128, 128]),
                        ("d_n2", n2b[:, 0:256], [128, 256]),
                        ("d_u2", u2[:], [128, 128]),
                        ("d_s2", s2[:], [128, 128]),
                    ]
                for dn, ap, shp in dump_specs:
                    dd = nc.dram_tensor(dn, shp, F32, kind="ExternalOutput")
                    db = wk.tile(shp, F32, tag=dn, name=dn)
                    nc.scalar.copy(db[:], ap)
                    nc.sync.dma_start(dd[:], db[:])

            # ---- final linear: outT = W_lin @ h2 + b_lin (gate-major) ----
            lin = r1b
            mm(lin[:, 0:64], wlin[0][:], s2[:, 0:64], start=True, stop=False, skip_group_check=True)
            mm(lin[:, 0:64], wlin[1][:], s2[:, 64:128], start=False, stop=False, skip_group_check=True)
            mm(lin[:, 0:64], blin[:], ones[:], start=False, stop=True, skip_group_check=True)
            osb = wk.tile([REP, BL], F32, tag="osb", name="osb")
            nc.scalar.copy(osb[:], lin[:, 0:64])
            nc.sync.dma_start(out_d[:], osb[:])

    nc.compile()
    return nc


def prep_inputs(input, cond, W_ih1, W_hh1, b_ih1, b_hh1, W_ih2, W_hh2,
                b_ih2, b_hh2, W_lin, b_lin, n_super=NSUPER):
    f = np.float32
    bf = bfloat16
    x = np.concatenate([np.asarray(input, f), np.asarray(cond, f)], axis=-1)

    W_ih1 = np.asarray(W_ih1, f); W_hh1 = np.asarray(W_hh1, f)
    b_ih1 = np.asarray(b_ih1, f); b_hh1 = np.asarray(b_hh1, f)
    W_ih2 = np.asarray(W_ih2, f); W_hh2 = np.asarray(W_hh2, f)
    b_ih2 = np.asarray(b_ih2, f); b_hh2 = np.asarray(b_hh2, f)

    Wih1T = W_ih1.T  # [80, 768]
    Whh1T = W_hh1.T  # [256, 768]
    Wih2T = W_ih2.T
    Whh2T = W_hh2.T

    wgi1 = np.zeros((DXA, 768), f)
    wgi1[0:80] = Wih1T
    wgi1[80, 0:512] = (b_ih1 + b_hh1)[0:512]
    wgi1[80, 512:768] = b_ih1[512:768]

    pat2 = np.zeros((2, 128), f)
    pat2[0, 0:64] = 1.0
    pat2[1, 64:128] = 1.0
    shared = {
        "wgi1": wgi1.astype(bf),
        "wgh1rz": Whh1T[:, 0:512].reshape(2, 128, 512).astype(bf),
        "wgh1n": Whh1T[:, 512:768].reshape(2, 128, 256).astype(bf),
        "bhh1n": b_hh1[512:768].reshape(2, 128).astype(bf),
        "wgi2rz": Wih2T[:, 0:512].reshape(2, 128, 512).astype(bf),
        "wgi2n": Wih2T[:, 512:768].reshape(2, 128, 256).astype(bf),
        "wgh2rz": Whh2T[:, 0:512].reshape(2, 128, 512).astype(bf),
        "wgh2n": Whh2T[:, 512:768].reshape(2, 128, 256).astype(bf),
        "br2": (b_ih2 + b_hh2)[0:256].reshape(2, 128).astype(bf),
        "bz2": (b_ih2 + b_hh2)[256:512].reshape(2, 128).astype(bf),
        "bin2": b_ih2[512:768].reshape(2, 128).astype(bf),
        "bhn2": b_hh2[512:768].reshape(2, 128).astype(bf),
        "wlin": np.asarray(W_lin, f).T.reshape(2, 128, REP).astype(bf),
        "blin": np.asarray(b_lin, f).reshape(1, REP).astype(bf),
        "ident": np.eye(128, dtype=f).astype(bf),
        "pat2": pat2.astype(bf),
        "ones": np.ones((1, BL), f).astype(bf),
    }

    in_maps = []
    for cidx in range(NCORES):
        xs = x[cidx * BL:(cidx + 1) * BL]            # [64, S, 80]
        xt = np.empty((DXA, S, BL), f)
        xt[0:80] = xs.transpose(2, 1, 0)
        xt[80] = 1.0
        m = dict(shared)
        m["xt"] = xt.astype(bf)
        in_maps.append(m)
    return in_maps


_program_cache = {}


def kernel(**inputs) -> np.ndarray:
    in_maps = prep_inputs(**inputs)
    if "nc" not in _program_cache:
        _program_cache["nc"] = build_program()
    nc = _program_cache["nc"]
    res = bass_utils.run_bass_kernel_spmd(nc, in_maps, core_ids=list(range(NCORES)))
    return np.concatenate(
        [np.asarray(r["outT"], np.float32).T for r in res.results], axis=0
    )
